# revision 8
# baseline (speedup 1.0000x reference)
"""TRN2 Bass kernel for nn_CombinedLoraA (moe_routing).

Computation: out[c, 0, r] = sum_k x[xids[c*64+r], 0, k] * A[wids[c], k, r]
  x: (512, 1, 4096) f32, xids: (20480,) i32, wids: (320,) i32, A: (80, 4096, 64) f32
  out: (320, 1, 64) f32

Strategy (adapter-parallel across 8 cores, routing baked in at trace time):
  - Host assigns exactly 10 adapters to each core (greedy row-count balance).
  - Each core computes the dense precompute P[w, t, r] = sum_k X[t, k] A[w, k, r]
    for ALL 512 tokens x its 10 adapters on the PE (X^T stationary, adapters'
    columns concatenated in the moving operand). X^T is transposed on the host.
  - P is copied PSUM->SBUF->DRAM per 128-token chunk (4 separate DRAM tables
    so gathers for chunk t overlap with chunk t+1's matmuls); the needed
    out[c, r] = P[w_c, tok[c, r], r] elements are fetched with indirect
    row-gathers (64 f32 per row), then a host-baked one-hot mask multiply +
    free-axis reduce picks the right column per (c, r) pair.
  - Host stitches the 8 per-core staging buffers into the (320, 1, 64) output.

Precision modes for the big matmul (error measured on HW at K=4096):
  float32 (3e-7), split bf16 hi/lo 3-matmul (4.5e-6), float32r (1.7e-4),
  bf16 (2.4e-3).
"""

import os
import sys

sys.path.insert(0, "/opt/trn_rl_repo")

import numpy as np
import ml_dtypes

import concourse.bass as bass
import concourse.tile as tile
from concourse import mybir, bacc
from concourse.bass import IndirectOffsetOnAxis
from concourse.bass_utils import run_bass_kernel_spmd

BATCH, C, R, K, NA = 512, 320, 64, 4096, 80
NCORES = 8
NW = NA // NCORES  # 10 adapters per core
KC = K // 128  # 32 contraction chunks
TC = BATCH // 128  # 4 token chunks
NFREE = NW * R  # 640 moving columns in the main matmul
PIECES = [1, 1] + [2] * 15  # kc-sized DMA pieces (sum = KC)

MODE = os.environ.get("KERNEL_MODE", "split")  # float32 | split | float32r | bf16
REPS = int(os.environ.get("BENCH_REPS", "1"))


def _plan(wids: np.ndarray):
    """Assign exactly NW adapters per core, balancing total row count."""
    rows_of = [[] for _ in range(NA)]
    for c, w in enumerate(wids):
        rows_of[w].append(c)
    order = sorted(range(NA), key=lambda w: -len(rows_of[w]))
    core_adapters = [[] for _ in range(NCORES)]
    core_load = [0] * NCORES
    for w in order:
        cands = [i for i in range(NCORES) if len(core_adapters[i]) < NW]
        i = min(cands, key=lambda j: core_load[j])
        core_adapters[i].append(w)
        core_load[i] += len(rows_of[w])
    return rows_of, core_adapters


def _mode_cfg():
    f32, bf16 = mybir.dt.float32, mybir.dt.bfloat16
    if MODE == "float32":
        return f32, 1, ((0, 512), (512, 640))
    if MODE == "float32r":
        return mybir.dt.float32r, 1, ((0, 320), (320, 640))
    if MODE == "bf16":
        return bf16, 1, ((0, 512), (512, 640))
    return bf16, 2, ((0, 512), (512, 640))  # split


def _build_bass(gchunks: list[int]):
    """gchunks[t] = number of 128-row gather chunks for token-chunk group t."""
    nc = bacc.Bacc("TRN2", target_bir_lowering=False, num_swdge_queues=2)
    f32 = mybir.dt.float32
    din, halves, chunks = _mode_cfg()
    nchunk = sum(gchunks)

    xt_d = [
        nc.dram_tensor(f"xt{h}", [K, BATCH], din, kind="ExternalInput")
        for h in range(halves)
    ]
    ac_d = [
        nc.dram_tensor(f"ac{h}", [KC, 128, NFREE], din, kind="ExternalInput")
        for h in range(halves)
    ]
    gcols = [gc * 8 for gc in gchunks]  # idx columns per group (num_idxs/16)
    gidx_d = nc.dram_tensor(
        "gidx", [128, sum(gcols)], mybir.dt.int16, kind="ExternalInput"
    )
    mask_d = nc.dram_tensor("mask", [128, nchunk, R], f32, kind="ExternalInput")
    out_d = nc.dram_tensor("out", [128, nchunk], f32, kind="ExternalOutput")

    with tile.TileContext(nc) as tc:
        with (
            tc.tile_pool(name="big", bufs=1) as big,
            tc.tile_pool(name="work", bufs=2) as work,
            tc.tile_pool(name="ps", bufs=1, space="PSUM") as ps,
            tc.tile_pool(name="dram", bufs=1, space="DRAM") as dpool,
        ):
            palls = [
                dpool.tile([128 * NW, R], f32, name=f"pall{t}") for t in range(TC)
            ]

            def body(_iv=None):
                # ---- resident loads, interleaved xt/ac pieces ----
                xts, acs = [], []
                for h in range(halves):
                    xts.append(
                        big.tile([128, KC, BATCH], din, tag=f"xt{h}", name=f"xt{h}")
                    )
                    acs.append(
                        big.tile([128, KC, NFREE], din, tag=f"ac{h}", name=f"ac{h}")
                    )
                off = 0
                for npc in PIECES:
                    sl = slice(off, off + npc)
                    for h in range(halves):
                        nc.sync.dma_start(
                            xts[h][:, sl, :],
                            xt_d[h].rearrange("(kc p) t -> p kc t", p=128)[:, sl, :],
                        )
                        nc.sync.dma_start(
                            acs[h][:, sl, :],
                            ac_d[h].rearrange("kc p n -> p kc n")[:, sl, :],
                        )
                    off += npc

                gidx = big.tile([128, sum(gcols)], mybir.dt.int16, name="gidx")
                nc.sync.dma_start(gidx[:], gidx_d[:])
                msk = big.tile([128, nchunk, R], f32, name="msk")
                nc.sync.dma_start(msk[:], mask_d[:])

                g = big.tile([128, nchunk, R], f32, name="g")
                tmp = big.tile([128, nchunk, R], f32, name="tmp")
                outsb = big.tile([128, nchunk], f32, name="outsb")

                # ---- main matmul: kc pieces interleaved across all tc so the
                # PE never waits on the tail of the input stream ----
                psts = [
                    [
                        ps.tile(
                            [128, hi - lo], f32, tag=f"ps{t}_{ci}", name=f"ps{t}_{ci}"
                        )
                        for ci, (lo, hi) in enumerate(chunks)
                    ]
                    for t in range(TC)
                ]
                off = 0
                for npc in PIECES:
                    for t in range(TC):
                        for kc in range(off, off + npc):
                            lhs = [xt[:, kc, t * 128 : (t + 1) * 128] for xt in xts]
                            rhs = [ac[:, kc, :] for ac in acs]
                            for ci, (lo, hi) in enumerate(chunks):
                                if halves == 1:
                                    nc.tensor.matmul(
                                        psts[t][ci][:],
                                        lhs[0],
                                        rhs[0][:, lo:hi],
                                        start=(kc == 0),
                                        stop=(kc == KC - 1),
                                    )
                                else:  # split: hi*hi + hi*lo + lo*hi
                                    for mi, (la, rb) in enumerate(
                                        ((0, 0), (0, 1), (1, 0))
                                    ):
                                        nc.tensor.matmul(
                                            psts[t][ci][:],
                                            lhs[la],
                                            rhs[rb][:, lo:hi],
                                            start=(kc == 0 and mi == 0),
                                            stop=(kc == KC - 1 and mi == 2),
                                        )
                    off += npc

                # ---- extraction: P dump -> gather -> mask multiply + reduce ----
                goff = 0
                for t in range(TC):
                    pcopy = work.tile([128, NFREE], f32, tag="pcopy", name="pcopy")
                    for ci, (lo, hi) in enumerate(chunks):
                        nc.vector.tensor_copy(pcopy[:, lo:hi], psts[t][ci][:])
                    nc.sync.dma_start(
                        palls[t][:].rearrange("(p w) r -> p (w r)", p=128), pcopy[:]
                    )
                    coff = sum(gcols[:t])
                    nidx = gchunks[t] * 128
                    nc.gpsimd.dma_gather(
                        out_ap=g[:, goff : goff + gchunks[t], :],
                        in_ap=palls[t][:],
                        idxs_ap=gidx[:, coff : coff + gcols[t]],
                        num_idxs=nidx,
                        num_idxs_reg=nidx,
                        elem_size=R,
                        queue_num=t % 2,
                    )
                    gsl = slice(goff, goff + gchunks[t])
                    nc.vector.tensor_tensor(
                        out=tmp[:, gsl, :],
                        in0=g[:, gsl, :],
                        in1=msk[:, gsl, :],
                        op=mybir.AluOpType.mult,
                    )
                    nc.vector.tensor_reduce(
                        out=outsb[:, gsl],
                        in_=tmp[:, gsl, :],
                        axis=mybir.AxisListType.X,
                        op=mybir.AluOpType.add,
                    )
                    goff += gchunks[t]

                nc.sync.dma_start(out_d[:], outsb[:])

            if REPS > 1:
                with tc.For_i(0, REPS, 1):
                    body()
            else:
                body()

    nc.compile()
    return nc


def _split_bf16(a: np.ndarray):
    hi = a.astype(ml_dtypes.bfloat16)
    lo = (a - hi.astype(np.float32)).astype(ml_dtypes.bfloat16)
    return hi, lo


def prepare(x, xids, wids, A):
    """Host-side planning + per-core input buffers. Returns (nc, in_maps, meta)."""
    x = np.ascontiguousarray(np.asarray(x).reshape(BATCH, K), dtype=np.float32)
    xids = np.asarray(xids).astype(np.int64)
    wids = np.asarray(wids).astype(np.int64)
    A = np.ascontiguousarray(np.asarray(A), dtype=np.float32)

    rows_of, core_adapters = _plan(wids)
    tok = xids.reshape(C, R)

    # per-core (c, r) pair lists grouped by token chunk, padded per group
    core_groups = []  # [core][t] -> list of (c, r)
    for core in range(NCORES):
        groups = [[] for _ in range(TC)]
        for w in core_adapters[core]:
            for c in rows_of[w]:
                for r in range(R):
                    groups[tok[c, r] // 128].append((c, r))
        core_groups.append(groups)
    gchunks = [
        max(1, max(-(-len(core_groups[core][t]) // 128) for core in range(NCORES)))
        for t in range(TC)
    ]
    nchunk = sum(gchunks)

    xt_f32 = np.ascontiguousarray(x.T)  # [K, BATCH]

    in_maps, pair_lists = [], []
    for core in range(NCORES):
        ws = core_adapters[core]
        acore = A[ws]  # [NW, K, R]
        ac_f32 = np.ascontiguousarray(acore.transpose(1, 0, 2).reshape(KC, 128, NFREE))

        slot = {w: i for i, w in enumerate(ws)}
        idx = np.zeros(nchunk * 128, dtype=np.int64)
        rvals = np.zeros(nchunk * 128, dtype=np.int64)
        pairs = np.full((nchunk * 128, 2), -1, dtype=np.int64)
        goff = 0
        gcol_blocks = []
        for t in range(TC):
            for i, (c, r) in enumerate(core_groups[core][t]):
                m = goff * 128 + i
                tt = tok[c, r]
                idx[m] = (tt % 128) * NW + slot[wids[c]]
                rvals[m] = r
                pairs[m] = (c, r)
            # wrapped int16 layout for this group: [16, nidx/16] -> tile rows
            nidx = gchunks[t] * 128
            gi = idx[goff * 128 : goff * 128 + nidx]
            blk = np.zeros((16, nidx // 16), dtype=np.int16)
            blk[np.arange(nidx) % 16, np.arange(nidx) // 16] = gi
            gcol_blocks.append(np.tile(blk, (8, 1)))  # replicate to 128 rows
            goff += gchunks[t]
        gidx = np.ascontiguousarray(np.concatenate(gcol_blocks, axis=1))
        mask = np.zeros((128, nchunk, R), dtype=np.float32)
        rv = rvals.reshape(nchunk, 128).T  # [128, nchunk]
        p_i, j_i = np.meshgrid(np.arange(128), np.arange(nchunk), indexing="ij")
        mask[p_i, j_i, rv] = 1.0

        m = {"gidx": gidx, "mask": mask}
        if MODE in ("float32", "float32r"):
            m["xt0"], m["ac0"] = xt_f32, ac_f32
        elif MODE == "bf16":
            m["xt0"] = xt_f32.astype(ml_dtypes.bfloat16)
            m["ac0"] = ac_f32.astype(ml_dtypes.bfloat16)
        else:  # split
            m["xt0"], m["xt1"] = _split_bf16(xt_f32)
            m["ac0"], m["ac1"] = _split_bf16(ac_f32)
        in_maps.append(m)
        pair_lists.append(pairs)

    nc = _build_bass(gchunks)
    return nc, in_maps, (pair_lists, nchunk)


def finish(results, meta):
    pair_lists, nchunk = meta
    out = np.zeros((C, 1, R), dtype=np.float32)
    for core in range(NCORES):
        vals = np.asarray(results[core]["out"]).T.reshape(-1)  # m = j*128+p
        pairs = pair_lists[core]
        sel = pairs[:, 0] >= 0
        out[pairs[sel, 0], 0, pairs[sel, 1]] = vals[sel]
    return out


def kernel(x, xids, wids, A):
    nc, in_maps, meta = prepare(x, xids, wids, A)
    res = run_bass_kernel_spmd(nc, in_maps, core_ids=list(range(NCORES)))
    return finish(res.results, meta)


if __name__ == "__main__":
    rng = np.random.default_rng(0)
    x = rng.standard_normal((BATCH, 1, K), dtype=np.float32)
    xids = rng.integers(0, BATCH, C * R).astype(np.int32)
    wids = rng.integers(0, NA, C).astype(np.int32)
    A = (rng.standard_normal((NA, K, R)) * 0.02).astype(np.float32)
    got = kernel(x=x, xids=xids, wids=wids, A=A)
    tokh = xids.reshape(C, R)
    want = np.einsum(
        "crk,ckr->cr",
        x[tokh, 0, :].astype(np.float64),
        A[wids].astype(np.float64),
    )[:, None, :]
    rel = np.abs(got - want).max() / np.abs(want).max()
    print(f"MODE={MODE} rel err vs f64: {rel:.3e}")


# revision 11
# speedup vs baseline: 224.0994x; 224.0994x over previous
"""TRN2 Bass kernel for nn_CombinedLoraA (moe_routing).

Computation: out[c, 0, r] = sum_k x[xids[c*64+r], 0, k] * A[wids[c], k, r]
  x: (512, 1, 4096) f32, xids: (20480,) i32, wids: (320,) i32, A: (80, 4096, 64) f32
  out: (320, 1, 64) f32

Strategy (adapter-parallel across 8 cores, routing baked in at trace time):
  - Host assigns exactly 10 adapters to each core (greedy row-count balance).
  - Each core computes the dense precompute P[w, t, r] = sum_k X[t, k] A[w, k, r]
    for ALL 512 tokens x its 10 adapters on the PE (X^T stationary, adapters'
    columns concatenated in the moving operand). X^T is transposed on the host.
  - P is copied PSUM->SBUF->DRAM per 128-token chunk (4 separate DRAM tables
    so gathers for chunk t overlap with chunk t+1's matmuls); the needed
    out[c, r] = P[w_c, tok[c, r], r] elements are fetched with indirect
    row-gathers (64 f32 per row), then a host-baked one-hot mask multiply +
    free-axis reduce picks the right column per (c, r) pair.
  - Host stitches the 8 per-core staging buffers into the (320, 1, 64) output.

Precision modes for the big matmul (error measured on HW at K=4096):
  float32 (3e-7), split bf16 hi/lo 3-matmul (4.5e-6), float32r (1.7e-4),
  bf16 (2.4e-3).
"""

import os
import sys

sys.path.insert(0, "/opt/trn_rl_repo")

import numpy as np
import ml_dtypes

import concourse.tile as tile
from concourse import mybir, bacc
from concourse.bass_utils import run_bass_kernel_spmd

BATCH, C, R, K, NA = 512, 320, 64, 4096, 80
NCORES = 8
NW = NA // NCORES  # 10 adapters per core
KC = K // 128  # 32 contraction chunks
TC = BATCH // 128  # 4 token chunks
NFREE = NW * R  # 640 moving columns in the main matmul
PIECES = [1, 1] + [2] * 15  # kc-sized DMA pieces (sum = KC)

SPLIT_KC = int(os.environ.get("SPLIT_KC", "20"))
MODE = os.environ.get("KERNEL_MODE", "split")  # float32 | split | float32r | bf16
REPS = int(os.environ.get("BENCH_REPS", "1"))


def _plan(wids: np.ndarray):
    """Assign exactly NW adapters per core, balancing total row count."""
    rows_of = [[] for _ in range(NA)]
    for c, w in enumerate(wids):
        rows_of[w].append(c)
    order = sorted(range(NA), key=lambda w: -len(rows_of[w]))
    core_adapters = [[] for _ in range(NCORES)]
    core_load = [0] * NCORES
    for w in order:
        cands = [i for i in range(NCORES) if len(core_adapters[i]) < NW]
        i = min(cands, key=lambda j: core_load[j])
        core_adapters[i].append(w)
        core_load[i] += len(rows_of[w])
    return rows_of, core_adapters


def _mode_cfg():
    f32, bf16 = mybir.dt.float32, mybir.dt.bfloat16
    if MODE == "float32":
        return f32, 1, ((0, 320), (320, 640))
    if MODE == "float32r":
        return mybir.dt.float32r, 1, ((0, 320), (320, 640))
    if MODE == "bf16":
        return bf16, 1, ((0, 320), (320, 640))
    return bf16, 2, ((0, 320), (320, 640))  # split


def _build_bass(gchunks: list[int]):
    """gchunks[t] = number of 128-row gather chunks for token-chunk group t."""
    nc = bacc.Bacc("TRN2", target_bir_lowering=False, num_swdge_queues=2)
    f32 = mybir.dt.float32
    din, halves, chunks = _mode_cfg()
    nchunk = sum(gchunks)

    xt_d = [
        nc.dram_tensor(f"xt{h}", [K, BATCH], din, kind="ExternalInput")
        for h in range(halves)
    ]
    ac_d = [
        nc.dram_tensor(f"ac{h}", [KC, 128, NFREE], din, kind="ExternalInput")
        for h in range(halves)
    ]
    gcols = [gc * 8 for gc in gchunks]  # idx columns per group (num_idxs/16)
    gidx_d = nc.dram_tensor(
        "gidx", [128, sum(gcols)], mybir.dt.int16, kind="ExternalInput"
    )
    mask_d = nc.dram_tensor("mask", [128, nchunk, R], f32, kind="ExternalInput")
    out_d = nc.dram_tensor("out", [128, nchunk], f32, kind="ExternalOutput")

    with tile.TileContext(nc) as tc:
        with (
            tc.tile_pool(name="big", bufs=1) as big,
            tc.tile_pool(name="work", bufs=2) as work,
            tc.tile_pool(name="ps", bufs=1, space="PSUM") as ps,
            tc.tile_pool(name="dram", bufs=1, space="DRAM") as dpool,
        ):
            palls = [
                dpool.tile([128 * NW, R], f32, name=f"pall{t}") for t in range(TC)
            ]

            def body(_iv=None):
                # ---- resident loads, interleaved xt/ac pieces ----
                xts, acs = [], []
                for h in range(halves):
                    xts.append(
                        big.tile([128, KC, BATCH], din, tag=f"xt{h}", name=f"xt{h}")
                    )
                    acs.append(
                        big.tile([128, KC, NFREE], din, tag=f"ac{h}", name=f"ac{h}")
                    )
                off = 0
                for npc in PIECES:
                    sl = slice(off, off + npc)
                    for h in range(halves):
                        nc.sync.dma_start(
                            xts[h][:, sl, :],
                            xt_d[h].rearrange("(kc p) t -> p kc t", p=128)[:, sl, :],
                        )
                        nc.sync.dma_start(
                            acs[h][:, sl, :],
                            ac_d[h].rearrange("kc p n -> p kc n")[:, sl, :],
                        )
                    off += npc

                gidx = big.tile([128, sum(gcols)], mybir.dt.int16, name="gidx")
                nc.sync.dma_start(gidx[:], gidx_d[:])
                msk = big.tile([128, nchunk, R], f32, name="msk")
                nc.sync.dma_start(msk[:], mask_d[:])

                g = big.tile([128, nchunk, R], f32, name="g")
                tmp = big.tile([128, nchunk, R], f32, name="tmp")
                outsb = big.tile([128, nchunk], f32, name="outsb")

                # ---- main matmul: kc pieces interleaved across all tc so the
                # PE never waits on the tail of the input stream ----
                psts = [
                    [
                        ps.tile(
                            [128, hi - lo], f32, tag=f"ps{t}_{ci}", name=f"ps{t}_{ci}"
                        )
                        for ci, (lo, hi) in enumerate(chunks)
                    ]
                    for t in range(TC)
                ]
                def emit_mms(t, kcs):
                    for kc in kcs:
                        lhs = [xt[:, kc, t * 128 : (t + 1) * 128] for xt in xts]
                        rhs = [ac[:, kc, :] for ac in acs]
                        for ci, (lo, hi) in enumerate(chunks):
                            if halves == 1:
                                nc.tensor.matmul(
                                    psts[t][ci][:],
                                    lhs[0],
                                    rhs[0][:, lo:hi],
                                    start=(kc == 0),
                                    stop=(kc == KC - 1),
                                )
                            else:  # split: hi*hi + hi*lo + lo*hi
                                for mi, (la, rb) in enumerate(((0, 0), (0, 1), (1, 0))):
                                    nc.tensor.matmul(
                                        psts[t][ci][:],
                                        lhs[la],
                                        rhs[rb][:, lo:hi],
                                        start=(kc == 0 and mi == 0),
                                        stop=(kc == KC - 1 and mi == 2),
                                    )

                # phase 1: kc pieces interleaved across all tc while the input
                # stream lands; phase 2: per-tc sequential so each tc finishes
                # early and its extraction overlaps the remaining matmuls
                off = 0
                for npc in PIECES:
                    if off >= SPLIT_KC:
                        break
                    for t in range(TC):
                        emit_mms(t, range(off, off + npc))
                    off += npc

                # ---- per-tc tail: mm -> P dump -> gather -> mask mul + reduce ----
                goff = 0
                for t in range(TC):
                    emit_mms(t, range(off, KC))
                    pcopy = work.tile([128, NFREE], f32, tag="pcopy", name="pcopy")
                    for ci, (lo, hi) in enumerate(chunks):
                        nc.vector.tensor_copy(pcopy[:, lo:hi], psts[t][ci][:])
                    nc.sync.dma_start(
                        palls[t][:].rearrange("(p w) r -> p (w r)", p=128), pcopy[:]
                    )
                    coff = sum(gcols[:t])
                    nidx = gchunks[t] * 128
                    nc.gpsimd.dma_gather(
                        out_ap=g[:, goff : goff + gchunks[t], :],
                        in_ap=palls[t][:],
                        idxs_ap=gidx[:, coff : coff + gcols[t]],
                        num_idxs=nidx,
                        num_idxs_reg=nidx,
                        elem_size=R,
                        queue_num=t % 2,
                    )
                    gsl = slice(goff, goff + gchunks[t])
                    nc.vector.tensor_tensor(
                        out=tmp[:, gsl, :],
                        in0=g[:, gsl, :],
                        in1=msk[:, gsl, :],
                        op=mybir.AluOpType.mult,
                    )
                    nc.vector.tensor_reduce(
                        out=outsb[:, gsl],
                        in_=tmp[:, gsl, :],
                        axis=mybir.AxisListType.X,
                        op=mybir.AluOpType.add,
                    )
                    goff += gchunks[t]

                nc.sync.dma_start(out_d[:], outsb[:])

            if REPS > 1:
                with tc.For_i(0, REPS, 1):
                    body()
            else:
                body()

    nc.compile()
    return nc


def _split_bf16(a: np.ndarray):
    hi = a.astype(ml_dtypes.bfloat16)
    lo = (a - hi.astype(np.float32)).astype(ml_dtypes.bfloat16)
    return hi, lo


def prepare(x, xids, wids, A):
    """Host-side planning + per-core input buffers. Returns (nc, in_maps, meta)."""
    x = np.ascontiguousarray(np.asarray(x).reshape(BATCH, K), dtype=np.float32)
    xids = np.asarray(xids).astype(np.int64)
    wids = np.asarray(wids).astype(np.int64)
    A = np.ascontiguousarray(np.asarray(A), dtype=np.float32)

    rows_of, core_adapters = _plan(wids)
    tok = xids.reshape(C, R)

    # per-core (c, r) pair lists grouped by token chunk, padded per group
    core_groups = []  # [core][t] -> list of (c, r)
    for core in range(NCORES):
        groups = [[] for _ in range(TC)]
        for w in core_adapters[core]:
            for c in rows_of[w]:
                for r in range(R):
                    groups[tok[c, r] // 128].append((c, r))
        core_groups.append(groups)
    gchunks = [
        max(1, max(-(-len(core_groups[core][t]) // 128) for core in range(NCORES)))
        for t in range(TC)
    ]
    nchunk = sum(gchunks)

    xt_f32 = np.ascontiguousarray(x.T)  # [K, BATCH]

    in_maps, pair_lists = [], []
    for core in range(NCORES):
        ws = core_adapters[core]
        acore = A[ws]  # [NW, K, R]
        ac_f32 = np.ascontiguousarray(acore.transpose(1, 0, 2).reshape(KC, 128, NFREE))

        slot = {w: i for i, w in enumerate(ws)}
        idx = np.zeros(nchunk * 128, dtype=np.int64)
        rvals = np.zeros(nchunk * 128, dtype=np.int64)
        pairs = np.full((nchunk * 128, 2), -1, dtype=np.int64)
        goff = 0
        gcol_blocks = []
        for t in range(TC):
            for i, (c, r) in enumerate(core_groups[core][t]):
                m = goff * 128 + i
                tt = tok[c, r]
                idx[m] = (tt % 128) * NW + slot[wids[c]]
                rvals[m] = r
                pairs[m] = (c, r)
            # wrapped int16 layout for this group: [16, nidx/16] -> tile rows
            nidx = gchunks[t] * 128
            gi = idx[goff * 128 : goff * 128 + nidx]
            blk = np.zeros((16, nidx // 16), dtype=np.int16)
            blk[np.arange(nidx) % 16, np.arange(nidx) // 16] = gi
            gcol_blocks.append(np.tile(blk, (8, 1)))  # replicate to 128 rows
            goff += gchunks[t]
        gidx = np.ascontiguousarray(np.concatenate(gcol_blocks, axis=1))
        mask = np.zeros((128, nchunk, R), dtype=np.float32)
        rv = rvals.reshape(nchunk, 128).T  # [128, nchunk]
        p_i, j_i = np.meshgrid(np.arange(128), np.arange(nchunk), indexing="ij")
        mask[p_i, j_i, rv] = 1.0

        m = {"gidx": gidx, "mask": mask}
        if MODE in ("float32", "float32r"):
            m["xt0"], m["ac0"] = xt_f32, ac_f32
        elif MODE == "bf16":
            m["xt0"] = xt_f32.astype(ml_dtypes.bfloat16)
            m["ac0"] = ac_f32.astype(ml_dtypes.bfloat16)
        else:  # split
            m["xt0"], m["xt1"] = _split_bf16(xt_f32)
            m["ac0"], m["ac1"] = _split_bf16(ac_f32)
        in_maps.append(m)
        pair_lists.append(pairs)

    nc = _build_bass(gchunks)
    return nc, in_maps, (pair_lists, nchunk)


def finish(results, meta):
    pair_lists, nchunk = meta
    out = np.zeros((C, 1, R), dtype=np.float32)
    for core in range(NCORES):
        vals = np.asarray(results[core]["out"]).T.reshape(-1)  # m = j*128+p
        pairs = pair_lists[core]
        sel = pairs[:, 0] >= 0
        out[pairs[sel, 0], 0, pairs[sel, 1]] = vals[sel]
    return out


def kernel(x, xids, wids, A):
    nc, in_maps, meta = prepare(x, xids, wids, A)
    res = run_bass_kernel_spmd(nc, in_maps, core_ids=list(range(NCORES)))
    return finish(res.results, meta)


if __name__ == "__main__":
    rng = np.random.default_rng(0)
    x = rng.standard_normal((BATCH, 1, K), dtype=np.float32)
    xids = rng.integers(0, BATCH, C * R).astype(np.int32)
    wids = rng.integers(0, NA, C).astype(np.int32)
    A = (rng.standard_normal((NA, K, R)) * 0.02).astype(np.float32)
    got = kernel(x=x, xids=xids, wids=wids, A=A)
    tokh = xids.reshape(C, R)
    want = np.einsum(
        "crk,ckr->cr",
        x[tokh, 0, :].astype(np.float64),
        A[wids].astype(np.float64),
    )[:, None, :]
    rel = np.abs(got - want).max() / np.abs(want).max()
    print(f"MODE={MODE} rel err vs f64: {rel:.3e}")
